# revision 11
# baseline (speedup 1.0000x reference)
"""Trainium2 Bass kernel for 16-head causal MHA with SSMax scaling + entropy.

Problem (hardcoded): B=4, S=2048, DIM=1024, H=16, DH=64.
reference: qkv = x @ Wqkv + bqkv; SSMax q-scale = scale_param[h]*ln(S);
scores = q.kT/sqrt(DH) masked-filled; softmax; out = (P@V) @ Wout + bout;
entropy = -(P*log(P+1e-9)).sum(-1).mean()  (returned as 2nd output).

Sharding (8 cores): core c -> batch b = c//2, head-group hg = c%2 (8 heads).
Each core computes qkv projection for its heads, causal flash-attention in a
"scores transposed" layout (k on partitions), the output projection partial
(its heads' contribution to out[b]), plus per-row softmax denominators Z and
score-weighted sums W for the entropy, staged out to DRAM. The host sums the
two partials per batch and computes entropy = mean(log Z - W/Z).

Device layout per head pair p (heads L=low partitions 0:64, H=high 64:128):
  qT/kT [128, S] f32r   (c on partitions; SSMax scale folded into Wq on host)
  vK    [128, S*2]      per k-tile 256 cols: [K_L|V_L | V_H|K_H]
  per (qc, kt): ST[k,q] psum [128, 2*QC] via 2 row-packed K=64 matmuls;
  causal mask added on the diagonal band from a precomputed wide mask tile;
  E = exp(ST) (f32r); per head one M=128 matmul lhsT=[K|V] accumulates
  KE (entropy dot operand) and V-out into one psum bank; M=1 ones-matmuls
  accumulate Z. Epilogue: recip(Z) broadcast via 0-stride DMA, V normalized
  into valuesT; W = ones.T @ (qhat*KE); Z,W staged to DRAM.
Out-projection: out_psum[st,mc] = sum_p valuesT_p[:,st].T @ Wout_p[:,mc]
  (+ rank-1 ones x bout term on the hg=0 core).
"""

import sys
import types

import numpy as np

B, S, DIM, H = 4, 2048, 1024, 16
DH = DIM // H
NC = 8
HG = H // 2  # heads per core
NPAIR = HG // 2  # head pairs per core
QC = 512  # q-chunk (psum bank width in fp32)
NQC = S // QC
NKT = S // 128
NDT = DIM // 128  # contraction d-tiles for projections

_CACHE = {}


def _install_env():
    """Make concourse importable + register the axon NTFF hook (idempotent)."""
    for p in ("/opt/trn_rl_repo", "/root/.axon_site/_ro/trn_rl_repo"):
        import os

        if os.path.isdir(p) and p not in sys.path:
            sys.path.append(p)
    if "antenv.axon_hooks" not in sys.modules:
        mod = types.ModuleType("antenv.axon_hooks")
        mod._hook = None
        mod.set_axon_ntff_profile_hook = lambda h: setattr(mod, "_hook", h)
        mod.get_axon_ntff_profile_hook = lambda: mod._hook
        sys.modules["antenv.axon_hooks"] = mod
    try:
        from trn_agent_boot.trn_boot import _ntff_profile_via_ctypes

        hook = _ntff_profile_via_ctypes("/opt/axon/libaxon_pjrt.so")
        sys.modules["antenv.axon_hooks"].set_axon_ntff_profile_hook(hook)
    except Exception:
        pass
    import concourse.bass_utils as bu

    bu.upload_artifacts = lambda tmpdir: "file://" + str(tmpdir)

    # Disk-cache walrus NEFF compiles keyed by the BIR hash so fresh
    # processes skip the multi-minute neuronx-cc step.
    import concourse.bass2jax as b2j

    if not getattr(b2j, "_neff_cache_installed", False):
        import hashlib
        import os
        import shutil

        cache_dir = os.environ.get("BASS_NEFF_CACHE", "/var/tmp/bass_neff_cache")
        orig = b2j.compile_bir_kernel

        def cached_compile(ant_bir_str, compile_dir_path, neff_name="file.neff"):
            h = hashlib.sha256(ant_bir_str).hexdigest()[:32]
            cpath = os.path.join(cache_dir, f"{h}_{neff_name}")
            dst = os.path.join(compile_dir_path, neff_name)
            if os.path.exists(cpath):
                shutil.copy(cpath, dst)
                return dst
            out = orig(ant_bir_str, compile_dir_path, neff_name=neff_name)
            try:
                os.makedirs(cache_dir, exist_ok=True)
                shutil.copy(out, cpath)
            except OSError:
                pass
            return out

        b2j.compile_bir_kernel = cached_compile
        b2j._neff_cache_installed = True


def build_program(mask_mode="causal", n_pairs=NPAIR, s=S, with_bias=True):
    """Build the per-core Bass program. mask_mode: causal | none."""
    _install_env()
    import concourse.tile as tile
    from concourse import bacc, mybir

    F32 = mybir.dt.float32
    F32R = mybir.dt.float32r
    EXP = mybir.ActivationFunctionType.Exp

    nqc = s // QC
    nkt = s // 128
    nst = s // 128

    nc = bacc.Bacc("TRN2", target_bir_lowering=False, debug=False, num_devices=NC)

    # ---- DRAM tensors ----
    xT = nc.dram_tensor("xT", [NDT, 128, s], F32R, kind="ExternalInput").ap()
    wqk = nc.dram_tensor("wqk", [n_pairs, NDT, 128, 256], F32R,
                         kind="ExternalInput").ap()
    wvk = nc.dram_tensor("wvk", [n_pairs, NDT, 128, 256], F32R,
                         kind="ExternalInput").ap()
    wo = nc.dram_tensor("wo", [n_pairs, 128, DIM], F32R, kind="ExternalInput").ap()
    bqv = nc.dram_tensor("bq", [128, n_pairs], F32, kind="ExternalInput").ap()
    bout = nc.dram_tensor("bout", [1, DIM], F32R, kind="ExternalInput").ap()
    maskw = nc.dram_tensor("maskw", [128, QC], F32, kind="ExternalInput").ap()
    onesd = nc.dram_tensor("ones", [128, 128], F32R, kind="ExternalInput").ap()
    seld = nc.dram_tensor("sel", [1, 256], F32R, kind="ExternalInput").ap()
    out_p = nc.dram_tensor("out_p", [s, DIM], F32, kind="ExternalOutput").ap()
    # per (pair, qc): [Z_L | Z_H | W_L | W_H] each QC wide
    zw_out = nc.dram_tensor("zw", [n_pairs * nqc, 4 * QC], F32R,
                            kind="ExternalOutput").ap()

    with tile.TileContext(nc) as tc:
        with (
            tc.tile_pool(name="big", bufs=1) as bigp,
            tc.tile_pool(name="wstream", bufs=NDT) as wsp,
            tc.tile_pool(name="qk", bufs=1) as qkp,
            tc.tile_pool(name="vals", bufs=n_pairs) as valp,
            tc.tile_pool(name="work", bufs=3) as wkp,
            tc.tile_pool(name="small", bufs=1) as smp,
            tc.tile_pool(name="stps", bufs=2, space="PSUM") as stp,
            tc.tile_pool(name="pvps", bufs=2, space="PSUM") as pvp,
            tc.tile_pool(name="zwps", bufs=1, space="PSUM") as zwp,
        ):
            # ---- persistent loads ----
            xT_sb = bigp.tile([128, NDT * s], F32R, tag="xT")
            for d in range(NDT):
                nc.sync.dma_start(xT_sb[:, d * s:(d + 1) * s], xT[d])
            ones_sb = bigp.tile([128, 128], F32R, tag="ones")
            nc.sync.dma_start(ones_sb[:], onesd[:])
            maskw_sb = bigp.tile([128, QC], F32, tag="maskw")
            nc.sync.dma_start(maskw_sb[:], maskw[:])
            bout_sb = bigp.tile([1, DIM], F32R, tag="bout")
            nc.sync.dma_start(bout_sb[:], bout[:])
            bq_sb = bigp.tile([128, n_pairs], F32, tag="bq")
            nc.sync.dma_start(bq_sb[:], bqv[:])
            sel_sb = bigp.tile([1, 256], F32R, tag="sel")
            nc.sync.dma_start(sel_sb[:], seld[:])

            vts = []
            for p in range(n_pairs):
                # ======== projections for pair p ========
                qT_p = qkp.tile([128, s], F32R, tag="qT", name=f"qT{p}")
                kT_p = qkp.tile([128, s], F32R, tag="kT", name=f"kT{p}")
                wqk_ts = []
                for d in range(NDT):
                    wt = wsp.tile([128, 256], F32R, tag="wqk", name=f"wqk{p}_{d}")
                    nc.sync.dma_start(wt[:], wqk[p, d])
                    wqk_ts.append(wt)
                for sch in range(s // 512):
                    pq = stp.tile([128, 1024], F32, tag="st", name=f"pq{p}_{sch}")
                    for d in range(NDT):
                        nc.tensor.matmul(
                            pq[:, 0:512], wqk_ts[d][:, 0:128],
                            xT_sb[:, d * s + sch * 512:d * s + (sch + 1) * 512],
                            start=(d == 0), stop=(d == NDT - 1),
                            skip_group_check=True)
                        nc.tensor.matmul(
                            pq[:, 512:1024], wqk_ts[d][:, 128:256],
                            xT_sb[:, d * s + sch * 512:d * s + (sch + 1) * 512],
                            start=(d == 0), stop=(d == NDT - 1),
                            skip_group_check=True)
                    if with_bias:
                        nc.vector.tensor_scalar_add(
                            qT_p[:, sch * 512:(sch + 1) * 512], pq[:, 0:512],
                            bq_sb[:, p:p + 1])
                    else:
                        nc.vector.tensor_copy(
                            qT_p[:, sch * 512:(sch + 1) * 512], pq[:, 0:512])
                    nc.scalar.copy(kT_p[:, sch * 512:(sch + 1) * 512],
                                   pq[:, 512:1024])
                vK_p = qkp.tile([128, nst * 256], F32R, tag="vK", name=f"vK{p}")
                wvk_ts = []
                for d in range(NDT):
                    wt = wsp.tile([128, 256], F32R, tag="wvk", name=f"wvk{p}_{d}")
                    nc.sync.dma_start(wt[:], wvk[p, d])
                    wvk_ts.append(wt)
                for st in range(nst):
                    pvk = pvp.tile([128, 256], F32, tag="pv", name=f"pvk{p}_{st}")
                    for d in range(NDT):
                        nc.tensor.matmul(
                            pvk[:],
                            xT_sb[:, d * s + st * 128:d * s + (st + 1) * 128],
                            wvk_ts[d][:], start=(d == 0), stop=(d == NDT - 1),
                            skip_group_check=True)
                    nc.scalar.copy(vK_p[:, st * 256:(st + 1) * 256], pvk[:])

                # ======== SDPA for pair p ========
                vt_p = valp.tile([128, s], F32R, tag="vt", name=f"vt{p}")
                vts.append(vt_p)
                for qc in range(nqc):
                    q0 = qc * QC
                    kt_hi = (q0 + QC) // 128 if mask_mode == "causal" else nkt
                    pvL = pvp.tile([128, QC], F32, tag="pv", name=f"pvL{p}_{qc}")
                    pvH = pvp.tile([128, QC], F32, tag="pv", name=f"pvH{p}_{qc}")
                    zps = zwp.tile([128, 1024], F32, tag="zw", name=f"z{p}_{qc}")
                    for kt in range(kt_hi):
                        st_ = stp.tile([128, 2 * QC], F32, tag="st",
                                       name=f"st{p}_{qc}_{kt}")
                        nc.tensor.matmul(
                            st_[:, 0:QC], kT_p[0:64, kt * 128:(kt + 1) * 128],
                            qT_p[0:64, q0:q0 + QC], start=True, stop=True,
                            tile_position=(0, 0))
                        nc.tensor.matmul(
                            st_[:, QC:2 * QC], kT_p[64:128, kt * 128:(kt + 1) * 128],
                            qT_p[64:128, q0:q0 + QC], start=True, stop=True,
                            tile_position=(64, 0))
                        if mask_mode == "causal":
                            r = kt * 128 - q0
                            if r >= 0:
                                w = r + 128
                                mw = maskw_sb[:, QC - w:QC]
                                nc.vector.tensor_add(st_[:, 0:w], st_[:, 0:w], mw)
                                nc.vector.tensor_add(st_[:, QC:QC + w],
                                                     st_[:, QC:QC + w], mw)
                        E = wkp.tile([128, 2 * QC], F32R, tag="E",
                                     name=f"E{p}_{qc}_{kt}", bufs=2)
                        nc.scalar.activation(E[:], st_[:], EXP)
                        first, last = kt == 0, kt == kt_hi - 1
                        base = kt * 256
                        # head L: lhsT [K_L|V_L] -> KE rows 0:64, V rows 64:128
                        nc.tensor.matmul(pvL[:], vK_p[:, base:base + 128],
                                         E[:, 0:QC], start=first, stop=last,
                                         skip_group_check=True)
                        # head H: lhsT [V_H|K_H] -> V rows 0:64, KE rows 64:128
                        nc.tensor.matmul(pvH[:], vK_p[:, base + 128:base + 256],
                                         E[:, QC:], start=first, stop=last,
                                         skip_group_check=True)
                        nc.tensor.matmul(zps[0:1, 0:QC], ones_sb[:, 0:1],
                                         E[:, 0:QC], start=first, stop=last,
                                         skip_group_check=True)
                        nc.tensor.matmul(zps[0:1, QC:2 * QC], ones_sb[:, 0:1],
                                         E[:, QC:], start=first, stop=last,
                                         skip_group_check=True)
                    # -------- epilogue (p, qc) --------
                    zwst = smp.tile([1, 2048], F32R, tag="zwst",
                                    name=f"zwst{p}_{qc}")
                    nc.scalar.copy(zwst[0:1, 0:1024], zps[0:1, 0:1024])
                    rz = smp.tile([1, 1024], F32R, tag="rz", name=f"rz{p}_{qc}")
                    with nc.allow_low_precision(reason="fp32r feeds PE bcast"):
                        nc.vector.reciprocal(rz[:], zps[0:1, 0:1024])
                    # broadcast recip(Z) across partitions via K=1 selector
                    # matmuls: rows 64:128 <- rz_L, rows 0:64 <- rz_H
                    bc = stp.tile([128, QC], F32, tag="st", name=f"bc{p}_{qc}")
                    nc.tensor.matmul(bc[:], sel_sb[0:1, 0:128], rz[0:1, 0:QC],
                                     start=True, stop=False,
                                     skip_group_check=True)
                    nc.tensor.matmul(bc[:], sel_sb[0:1, 128:256],
                                     rz[0:1, QC:2 * QC], start=False, stop=True,
                                     skip_group_check=True)
                    rzb = wkp.tile([128, QC], F32, tag="rzb",
                                   name=f"rzb{p}_{qc}", bufs=2)
                    nc.scalar.copy(rzb[:], bc[:])
                    nc.vector.tensor_mul(vt_p[64:128, q0:q0 + QC], pvL[64:128, :],
                                         rzb[64:128, :])
                    nc.vector.tensor_mul(vt_p[0:64, q0:q0 + QC], pvH[0:64, :],
                                         rzb[0:64, :])
                    qE = wkp.tile([128, QC], F32R, tag="qE",
                                  name=f"qE{p}_{qc}", bufs=2)
                    nc.vector.tensor_mul(qE[0:64, :], qT_p[0:64, q0:q0 + QC],
                                         pvL[0:64, :])
                    nc.vector.tensor_mul(qE[64:128, :], qT_p[64:128, q0:q0 + QC],
                                         pvH[64:128, :])
                    wps = zwp.tile([128, 1024], F32, tag="zw", name=f"w{p}_{qc}")
                    nc.tensor.matmul(wps[0:1, 0:QC], ones_sb[0:64, 0:1],
                                     qE[0:64, :], start=True, stop=True,
                                     tile_position=(0, 0))
                    nc.tensor.matmul(wps[0:1, QC:2 * QC], ones_sb[64:128, 0:1],
                                     qE[64:128, :], start=True, stop=True,
                                     tile_position=(64, 0))
                    nc.scalar.copy(zwst[0:1, 1024:2048], wps[0:1, 0:1024])
                    nc.sync.dma_start(zw_out[p * nqc + qc:p * nqc + qc + 1, :],
                                      zwst[:])

            # ======== output projection (mc outer, wo streamed by half) ========
            for mc in range(DIM // 512):
                wo_sbs = []
                for p in range(n_pairs):
                    wt = wsp.tile([128, 512], F32R, tag="wo",
                                  name=f"wo{p}_{mc}", bufs=n_pairs)
                    nc.sync.dma_start(wt[:], wo[p, :, mc * 512:(mc + 1) * 512])
                    wo_sbs.append(wt)
                for st in range(nst):
                    po = pvp.tile([128, 512], F32, tag="pv", name=f"po{st}_{mc}")
                    for p in range(n_pairs):
                        nc.tensor.matmul(
                            po[:], vts[p][:, st * 128:(st + 1) * 128],
                            wo_sbs[p][:],
                            start=(p == 0), stop=False, skip_group_check=True)
                    nc.tensor.matmul(po[:], ones_sb[0:1, 0:128],
                                     bout_sb[0:1, mc * 512:(mc + 1) * 512],
                                     start=False, stop=True,
                                     skip_group_check=True)
                    osb = wkp.tile([128, 512], F32, tag="osb",
                                   name=f"osb{st}_{mc}", bufs=2)
                    nc.vector.tensor_copy(osb[:], po[:])
                    nc.sync.dma_start(
                        out_p[st * 128:(st + 1) * 128, mc * 512:(mc + 1) * 512],
                        osb[:])
    nc.finalize()
    return nc


def prep_core_inputs(core, x, Wqkv, bqkv, Wout, bout, scale_param, mask_mode):
    """Host-side shard prep for one core -> in_map dict of np arrays."""
    b, hg = core // 2, core % 2
    h0 = hg * HG
    lnS = np.log(np.float32(S))
    inv_sqrt_dh = 1.0 / np.sqrt(np.float32(DH))

    xT = np.ascontiguousarray(x[b].T)  # [DIM, S]
    xTr = xT.reshape(NDT, 128, S)

    wqk = np.zeros((NPAIR, DIM, 256), np.float32)
    wvk = np.zeros((NPAIR, DIM, 256), np.float32)
    wo = np.zeros((NPAIR, 128, DIM), np.float32)
    bq = np.zeros((128, NPAIR), np.float32)
    for p in range(NPAIR):
        hL, hH = h0 + 2 * p, h0 + 2 * p + 1
        for half, h in ((0, hL), (1, hH)):
            alpha = (scale_param[h] * lnS * inv_sqrt_dh).astype(np.float32)
            # reference reshapes [.., 3*DIM] -> [.., H, 3*DH]: head h owns
            # columns [h*3DH, (h+1)*3DH) split as q|k|v of DH each.
            base = h * 3 * DH
            qcols = Wqkv[:, base:base + DH] * alpha
            kcols = Wqkv[:, base + DH:base + 2 * DH]
            vcols = Wqkv[:, base + 2 * DH:base + 3 * DH]
            wqk[p, :, half * 64:half * 64 + 64] = qcols
            wqk[p, :, 128 + half * 64:128 + half * 64 + 64] = kcols
            bq[half * 64:half * 64 + 64, p] = bqkv[base:base + DH] * alpha
            if half == 0:  # L: [K_L | V_L]
                wvk[p, :, 0:64] = kcols
                wvk[p, :, 64:128] = vcols
            else:  # H: [V_H | K_H]
                wvk[p, :, 128:192] = vcols
                wvk[p, :, 192:256] = kcols
        # valuesT rows: 0:64 = head H, 64:128 = head L
        wo[p, 0:64, :] = Wout[hH * DH:(hH + 1) * DH, :]
        wo[p, 64:128, :] = Wout[hL * DH:(hL + 1) * DH, :]

    # v-bias in value-dim order (dim index = h*DH + d)
    bv = np.concatenate(
        [bqkv[h * 3 * DH + 2 * DH:h * 3 * DH + 3 * DH] for h in range(H)])
    bout_eff = (bout + bv @ Wout).astype(np.float32)
    bout_core = bout_eff if hg == 0 else np.zeros_like(bout_eff)

    maskw = np.zeros((128, QC), np.float32)
    maskw[:, 0:QC - 128] = -1e30
    tri = np.tril(np.full((128, 128), -1e30, np.float32), -1)  # mask k>q
    maskw[:, QC - 128:QC] = tri

    return {
        "xT": xTr,
        "wqk": wqk.reshape(NPAIR, NDT, 128, 256),
        "wvk": wvk.reshape(NPAIR, NDT, 128, 256),
        "wo": wo,
        "bq": bq,
        "bout": bout_core.reshape(1, DIM),
        "maskw": maskw,
        "ones": np.ones((128, 128), np.float32),
        "sel": np.concatenate([
            (np.arange(128) >= 64).astype(np.float32),
            (np.arange(128) < 64).astype(np.float32),
        ]).reshape(1, 256),
    }


def kernel(x, Wqkv, bqkv, Wout, bout, scale_param, mask):
    _install_env()
    from concourse import bass_utils

    x = np.asarray(x, np.float32)
    Wqkv = np.asarray(Wqkv, np.float32)
    bqkv = np.asarray(bqkv, np.float32)
    Wout = np.asarray(Wout, np.float32)
    bout = np.asarray(bout, np.float32)
    scale_param = np.asarray(scale_param, np.float32)
    mask = np.asarray(mask)

    causal = bool(
        np.array_equal(mask, np.triu(np.ones((S, S), bool), 1)))
    if causal:
        mask_mode = "causal"
    elif not mask.any():
        mask_mode = "none"
    else:
        raise NotImplementedError("general mask not supported")

    key = mask_mode
    if key not in _CACHE:
        _CACHE[key] = build_program(mask_mode=mask_mode)
    nc = _CACHE[key]

    in_maps = [
        prep_core_inputs(c, x, Wqkv, bqkv, Wout, bout, scale_param, mask_mode)
        for c in range(NC)
    ]
    res = bass_utils.run_bass_kernel_spmd(nc, in_maps, core_ids=list(range(NC)))

    out = np.zeros((B, S, DIM), np.float32)
    ent_sum = 0.0
    for c in range(NC):
        b = c // 2
        out[b] += res.results[c]["out_p"]
        zw = res.results[c]["zw"].astype(np.float64)  # [NPAIR*NQC, 2048]
        Z = np.concatenate([zw[:, 0:1024].reshape(-1, QC)], 0)
        W = np.concatenate([zw[:, 1024:2048].reshape(-1, QC)], 0)
        ent_sum += float((np.log(Z) - W / Z).sum())
    entropy = np.float32(ent_sum / (B * H * S))
    return out, entropy
